# revision 1
# baseline (speedup 1.0000x reference)
"""NeuroPlasticLite Trainium2 kernel (8-core data-parallel over batch).

Math (per core, batch shard BS=64):
  rows r = (b, n), b in [0,64), n in [0,256).
  x-space layout: SBUF [128 partitions = nlo, free = fc*32 + d] where
  fc = nhi*64 + b, n = nhi*128 + nlo.  (p = nlo makes the activation
  tensor a[b, m] naturally contraction-partitioned for the syn matmul.)

  Loop (20 steps), x kept in SBUF, updated via
    x <- c1*x + PSUM(h-matmuls + V-pass)
  where h = DT*(gelu(w1*syn + b1) @ w2.T), V = DT*(u@w_in.T + bias + b2),
  c1 = 1 - DT*GAMMA.  syn = a @ W_sp with W_sp the dense scatter of the
  top-50 cosine-sim weights (computed host-side, replicated).
"""

import os
from contextlib import ExitStack

import numpy as np

N, D, KF, KN = 256, 32, 16, 50
GAMMA, LAM_A, DT, STEPS = 0.1, 0.95, 0.05, 20
B, UIN = 512, 128
NCORES = 8
BS = B // NCORES          # 64 batch rows per core
R = BS * N                # 16384 rows per core
C1 = 1.0 - DT * GAMMA     # 0.995
EPS = 1e-12

_cache = {}


def _host_prep(features, bias, w_in, b_in, sig_w1, sig_b1, sig_w2, sig_b2):
    """All tiny, replicated tensors, as numpy (fp32)."""
    f = features / np.linalg.norm(features, axis=1, keepdims=True)
    sim = f @ f.T                                   # [N, N]
    # top-KN per row (order irrelevant; ties vanishingly unlikely)
    idx = np.argsort(-sim, axis=1, kind="stable")[:, :KN]        # [N, KN]
    vals = np.take_along_axis(sim, idx, axis=1)                  # [N, KN]
    W = np.zeros((N, N), np.float32)                             # W[m, n]
    np.add.at(W, (idx, np.arange(N)[:, None]), vals)

    # syn-matmul lhsT blocks: wt[:, (mhi*2+nhi)*128 : +128][mlo, nlo]
    #   = W[mhi*128+mlo, nhi*128+nlo]
    wt = np.concatenate(
        [W[mh * 128:(mh + 1) * 128, nh * 128:(nh + 1) * 128]
         for mh in (0, 1) for nh in (0, 1)], axis=1,
    ).astype(np.float32)                                         # [128, 512]

    winT = (DT * w_in.T).astype(np.float32)                      # [128, 32]
    # u_proj uses u @ w_in.T + b_in ; b_in is part of the loop-invariant V
    # biasSm[p, nhi*32 + d] = DT*(bias[nhi*128+p, d] + b_in[d] + sig_b2[d])
    badd = bias + b_in[None, :] + sig_b2[None, :]                # [256, 32]
    biasSm = np.concatenate(
        [DT * badd[0:128, :], DT * badd[128:256, :]], axis=1
    ).astype(np.float32)                                         # [128, 64]

    # h-matmul rhs: block-diag bd[(a8, j16), (a'8, d32)] = d(a==a')*DT*w2[d, j]
    bd = np.zeros((128, 256), np.float32)
    for a in range(8):
        # rows a*16..a*16+16 (j), cols a*32..a*32+32 (d)
        bd[a * 16:(a + 1) * 16, a * 32:(a + 1) * 32] = DT * sig_w2.T
    # round bd to tf32 (f32r) so the f32r h-matmul consumes exact values
    bdi = bd.view(np.uint32)
    bdi &= np.uint32(0xFFFFE000)
    ident = np.eye(128, dtype=np.float32)

    w1 = [float(sig_w1[j, 0]) for j in range(16)]
    b1 = [float(sig_b1[j]) for j in range(16)]
    return wt, winT, biasSm, bd, ident, w1, b1


def build_nc(w1, b1, n_cores):
    import concourse.bacc as bacc
    import concourse.tile as tile
    from concourse import mybir

    f32 = mybir.dt.float32
    f32r = mybir.dt.float32r
    AF = mybir.ActivationFunctionType
    OP = mybir.AluOpType
    AX = mybir.AxisListType

    nc = bacc.Bacc("TRN2", target_bir_lowering=False, debug=False,
                   num_devices=n_cores)
    u_s = nc.declare_dram_parameter("u_s", [R, UIN], f32, isOutput=False)
    wt_d = nc.declare_dram_parameter("wt", [128, 512], f32, isOutput=False)
    winT_d = nc.declare_dram_parameter("winT", [128, 32], f32, isOutput=False)
    biasSm_d = nc.declare_dram_parameter("biasSm", [128, 64], f32, isOutput=False)
    bd_d = nc.declare_dram_parameter("bd", [128, 256], f32r, isOutput=False)
    ident_d = nc.declare_dram_parameter("ident", [128, 128], f32, isOutput=False)
    xout = nc.declare_dram_parameter("xout", [128, 4096], f32, isOutput=True)

    def r32(ap):
        return ap.bitcast(f32r)

    with tile.TileContext(nc) as tc:
        with ExitStack() as ctx:
            cpool = ctx.enter_context(tc.tile_pool(name="consts", bufs=1))
            wt = cpool.tile([128, 512], f32)
            nc.sync.dma_start(wt[:], wt_d[:])
            winT = cpool.tile([128, 32], f32)
            nc.sync.dma_start(winT[:], winT_d[:])
            biasSm = cpool.tile([128, 64], f32)
            nc.sync.dma_start(biasSm[:], biasSm_d[:])
            bd = cpool.tile([128, 256], f32r)
            nc.sync.dma_start(bd[:], bd_d[:])
            ident = cpool.tile([128, 128], f32)
            nc.sync.dma_start(ident[:], ident_d[:])
            eps_t = cpool.tile([128, 1], f32)
            nc.vector.memset(eps_t[:], EPS)

            spool = ctx.enter_context(tc.tile_pool(name="state", bufs=1))
            x_sb = spool.tile([128, 4096], f32)
            V_sb = spool.tile([128, 4096], f32)
            G_sb = spool.tile([128, 2048], f32)

            # ---------- Phase A: u_proj -> V ----------
            with ExitStack() as actx:
                upool = actx.enter_context(tc.tile_pool(name="u", bufs=3))
                utp = actx.enter_context(
                    tc.tile_pool(name="utp", bufs=3, space="PSUM"))
                utsp = actx.enter_context(tc.tile_pool(name="uts", bufs=4))
                vpp = actx.enter_context(
                    tc.tile_pool(name="vp", bufs=2, space="PSUM"))

                def assemble_V(bank, vb):
                    nhi = (bank * 16) // 64
                    bsl = biasSm[:, nhi * 32:(nhi + 1) * 32]
                    brd = bsl.unsqueeze(1).broadcast_to((128, 16, 32))
                    nc.vector.tensor_tensor(
                        V_sb[:, bank * 512:(bank + 1) * 512].rearrange(
                            "p (s d) -> p s d", d=32),
                        vb[:].rearrange("p (s d) -> p s d", d=32),
                        brd, op=OP.add)

                for g in range(4):                     # bank pair (g, 4+g)
                    vlo = vpp.tile([128, 512], f32, tag="vlo")
                    vhi = vpp.tile([128, 512], f32, tag="vhi")
                    for c in range(4 * g, 4 * g + 4):  # u chunks of 1024 rows
                        uch = upool.tile([128, 1024], f32)
                        nc.sync.dma_start(
                            uch[:].rearrange("p (s k) -> p s k", k=128),
                            u_s[1024 * c:1024 * (c + 1), :].rearrange(
                                "(s p) k -> p s k", p=128))
                        for sb in range(8):
                            i = 8 * c + sb
                            fc = (i % 2) * 64 + i // 2
                            vb = vlo if fc < 64 else vhi
                            slot = fc % 16
                            tp = utp.tile([128, 128], f32)
                            nc.tensor.transpose(
                                tp[:], uch[:, sb * 128:(sb + 1) * 128],
                                ident[:])
                            uts = utsp.tile([128, 128], f32)
                            nc.scalar.copy(uts[:], tp[:])
                            nc.tensor.matmul(
                                vb[:, slot * 32:(slot + 1) * 32], uts[:],
                                winT[:], start=True, stop=True)
                    assemble_V(g, vlo)
                    assemble_V(4 + g, vhi)

            nc.vector.memset(x_sb[:], 0.0)

            # ---------- Phase B: 20 steps ----------
            lpool = ctx.enter_context(tc.tile_pool(name="loop", bufs=2))
            spsum = ctx.enter_context(
                tc.tile_pool(name="spsum", bufs=1, space="PSUM"))
            tpsum = ctx.enter_context(
                tc.tile_pool(name="tpsum", bufs=2, space="PSUM"))
            xpsum = ctx.enter_context(
                tc.tile_pool(name="xpsum", bufs=1, space="PSUM"))
            tsp = ctx.enter_context(tc.tile_pool(name="ts", bufs=3))

            for t in range(STEPS):
                xsq = lpool.tile([128, 4096], f32, tag="xsq")
                nsq = lpool.tile([128, 128], f32, tag="nsq")
                for ch in range(4):
                    csl = slice(ch * 1024, (ch + 1) * 1024)
                    if ch % 2 == 0:
                        nc.scalar.activation(
                            xsq[:, csl], x_sb[:, csl], AF.Square)
                    else:
                        nc.vector.tensor_tensor(
                            xsq[:, csl], x_sb[:, csl], x_sb[:, csl],
                            op=OP.mult)
                    nc.vector.reduce_sum(
                        nsq[:, ch * 32:(ch + 1) * 32],
                        xsq[:, csl].rearrange("p (f d) -> p f d", d=32),
                        axis=AX.X)
                nrm = lpool.tile([128, 128], f32, tag="nrm")
                nc.scalar.activation(nrm[:], nsq[:], AF.Sqrt, bias=eps_t[:, 0:1])
                a_sb = lpool.tile([128, 128], f32, tag="a")
                nc.scalar.activation(a_sb[:], nrm[:], AF.Tanh)

                syn = spsum.tile([128, 128], f32, tag="syn")
                for nh in (0, 1):
                    for mh in (0, 1):
                        nc.tensor.matmul(
                            syn[:, nh * 64:(nh + 1) * 64],
                            wt[:, (mh * 2 + nh) * 128:(mh * 2 + nh + 1) * 128],
                            a_sb[:, mh * 64:(mh + 1) * 64],
                            start=(mh == 0), stop=(mh == 1))

                for j in range(16):
                    nc.scalar.activation(
                        G_sb[:, j:2048:16], syn[:], AF.Gelu,
                        bias=b1[j], scale=w1[j])

                for half in (0, 1):
                    xp = xpsum.tile([128, 2048], f32, tag="xp")
                    for c in range(4):
                        nc.tensor.matmul(
                            xp[:, c * 512:(c + 1) * 512], ident[:],
                            V_sb[:, half * 2048 + c * 512:
                                 half * 2048 + (c + 1) * 512],
                            start=True, stop=False, skip_group_check=True)
                    for q in (0, 1):                 # quads of 4 transposes
                        tp4 = tpsum.tile([128, 512], f32, tag="gt4")
                        for o4 in range(4):
                            O = half * 8 + q * 4 + o4
                            nc.tensor.transpose(
                                tp4[:, o4 * 128:(o4 + 1) * 128],
                                G_sb[:, O * 128:(O + 1) * 128], ident[:])
                        ts4 = tsp.tile([128, 512], f32r, tag="ts4")
                        nc.scalar.copy(ts4[:], tp4[:])
                        for o4 in range(4):
                            o = q * 4 + o4
                            nc.tensor.matmul(
                                xp[:, o * 256:(o + 1) * 256],
                                ts4[:, o4 * 128:(o4 + 1) * 128], bd[:],
                                start=False, stop=(o == 7),
                                skip_group_check=True)
                    sl = slice(half * 2048, (half + 1) * 2048)
                    nc.vector.scalar_tensor_tensor(
                        x_sb[:, sl], x_sb[:, sl], C1, xp[:],
                        op0=OP.mult, op1=OP.add)

            # ---------- Phase C: output ----------
            nc.sync.dma_start(xout[:], x_sb[:])
    nc.finalize()
    return nc


def _get_nc(key, w1, b1, n_cores):
    if key not in _cache:
        _cache[key] = build_nc(w1, b1, n_cores)
    return _cache[key]


def kernel(u, features, bias, w_in, b_in, sig_w1, sig_b1, sig_w2, sig_b2):
    from concourse.bass_utils import run_bass_kernel_spmd

    u = np.asarray(u, np.float32)
    args = [np.asarray(a, np.float32) for a in
            (features, bias, w_in, b_in, sig_w1, sig_b1, sig_w2, sig_b2)]
    wt, winT, biasSm, bd, ident, w1, b1 = _host_prep(*args)

    key = (tuple(w1), tuple(b1))
    nc = _get_nc(key, w1, b1, NCORES)

    in_maps = []
    for c in range(NCORES):
        u_shard = np.ascontiguousarray(
            u[c * BS:(c + 1) * BS].reshape(R, UIN))
        in_maps.append({
            "u_s": u_shard, "wt": wt, "winT": winT,
            "biasSm": biasSm, "bd": bd, "ident": ident,
        })
    res = run_bass_kernel_spmd(nc, in_maps, list(range(NCORES)))

    out = np.empty((B, N, D), np.float32)
    for c in range(NCORES):
        xo = res.results[c]["xout"]                  # [128, 4096]
        # xo[nlo, fc*32+d]; fc = nhi*64+b, n = nhi*128+nlo
        v = xo.reshape(128, 2, 64, 32)               # [nlo, nhi, b, d]
        out[c * BS:(c + 1) * BS] = (
            v.transpose(2, 1, 0, 3).reshape(BS, N, D))
    return out

